# revision 8
# baseline (speedup 1.0000x reference)
"""Trainium2 Bass kernel for nn_Decoder_10222022164898.

Math (reference):
    t      = trg @ fc_w.T + fc_b          # [B, L, H]
    norm1  = ||t||_2 over H               # [B, L]
    w      = softmax(norm1, axis=L)       # [B, L]
    summ   = sum_l w[l] * t[l, :]         # [B, H]
    return (summ, norm1)

Strategy: pure data-parallel over batch B=32 across 8 NeuronCores (4
batches/core, softmax is over L which stays core-local -> no collectives).

Per core, single pass over L:
  - host pre-transposes trg -> trgT [b, K, L] (bf16) so the contraction dim K
    lands on SBUF partitions with natural DMA.
  - per 128-row l-tile: PE accumulates t = trgT_tile.T @ fc_wT into PSUM over
    24 k-tiles (bf16), + one K=1 matmul ones(1x128) x fc_b for the bias.
  - ACT copies PSUM->SBUF (fp32 t), DVE does fused square+reduce straight
    from PSUM -> sumsq, ACT sqrt -> norm1, ACT exp(norm1-32) -> e.
  - PE accumulates e.T @ t into a persistent PSUM (pooling numerator);
    softmax normalization deferred to batch end:  summ = (sum_l e_l t_l) / Z.
  - batch end: Z via ones.T @ E matmul + reduce, reciprocal, scale, DMA out.
"""

import numpy as np
import ml_dtypes

import concourse.bass as bass
import concourse.mybir as mybir
import concourse.tile as tile
from concourse import bacc
from concourse.bass_utils import run_bass_kernel_spmd

F32 = mybir.dt.float32
BF16 = mybir.dt.bfloat16
F32R = mybir.dt.float32r
ALU = mybir.AluOpType
ACTF = mybir.ActivationFunctionType
AX = mybir.AxisListType

# Problem shapes (hardcoded; harness contract)
B, L, K, H = 32, 2048, 3072, 1024
NCORES = 8
BPC = B // NCORES          # batches per core
C_SHIFT = 32.0             # softmax stabilization shift (E||t|| = sqrt(H) = 32)


class Cfg:
    def __init__(self, bpc=BPC, l=L, k=K, h=H, l_super=512):
        assert l % l_super == 0 and l_super % 128 == 0
        assert k % 128 == 0 and h % 512 == 0
        self.bpc, self.l, self.k, self.h = bpc, l, k, h
        self.l_super = l_super
        self.kt = k // 128          # k-tiles
        self.lt = l // 128          # l-tiles per batch
        self.nsup = l // l_super    # DMA supertiles per batch
        self.ltps = l_super // 128  # l-tiles per supertile
        self.nh = h // 512          # h chunks of 512


def emit_kernel(tc: tile.TileContext, cfg: Cfg, ctx):
    nc = tc.nc
    P = 128
    NH = cfg.nh

    trgT = nc.dram_tensor("trgT", [cfg.bpc, cfg.k, cfg.l], BF16, kind="ExternalInput").ap()
    fcwT = nc.dram_tensor("fcwT", [cfg.k, cfg.h], BF16, kind="ExternalInput").ap()
    fcb = nc.dram_tensor("fcb", [1, cfg.h], BF16, kind="ExternalInput").ap()
    summ_o = nc.dram_tensor("summ", [cfg.bpc, cfg.h], F32, kind="ExternalOutput").ap()
    norm_o = nc.dram_tensor("norm1", [cfg.bpc, cfg.l], F32, kind="ExternalOutput").ap()

    const = ctx.enter_context(tc.tile_pool(name="const", bufs=1))
    a_pool = ctx.enter_context(tc.tile_pool(name="a_pool", bufs=2))
    t_pool = ctx.enter_context(tc.tile_pool(name="t_pool", bufs=3))
    sq_pool = ctx.enter_context(tc.tile_pool(name="sq_pool", bufs=2))
    stat = ctx.enter_context(tc.tile_pool(name="stat", bufs=2))
    small = ctx.enter_context(tc.tile_pool(name="small", bufs=2))
    psum_t = ctx.enter_context(tc.tile_pool(name="psum_t", bufs=2, space="PSUM"))
    psum_s = ctx.enter_context(tc.tile_pool(name="psum_s", bufs=1, space="PSUM"))
    psum_z = ctx.enter_context(tc.tile_pool(name="psum_z", bufs=1, space="PSUM"))

    # --- constants resident in SBUF ---
    w_sb = const.tile([P, cfg.kt, cfg.h], BF16, name="w_sb", tag="w_sb")
    w_src = fcwT.rearrange("(ko ki) h -> ki ko h", ki=P)
    # split the weight load across a few DMA queues
    for ko in range(cfg.kt):
        nc.sync.dma_start(w_sb[:, ko, :], w_src[:, ko, :])
    fcb_sb = const.tile([1, cfg.h], BF16, name="fcb_sb", tag="fcb_sb")
    nc.sync.dma_start(fcb_sb[:], fcb[:])
    ones_row = const.tile([1, P], BF16, name="ones_row", tag="ones_row")
    nc.vector.memset(ones_row[:], 1.0)
    ones_f32 = const.tile([P, 1], F32, name="ones_f32", tag="ones_f32")
    nc.vector.memset(ones_f32[:], 1.0)
    ones_col = const.tile([P, 1], F32R, name="ones_col", tag="ones_col")
    nc.scalar.copy(ones_col[:], ones_f32[:])
    negc = const.tile([P, 1], F32, name="negc", tag="negc")
    nc.vector.memset(negc[:], -C_SHIFT)

    for b in range(cfg.bpc):
        E_all = stat.tile([P, cfg.lt], F32R, name="E_all", tag="E_all")
        N_all = stat.tile([P, cfg.lt], F32, name="N_all", tag="N_all")
        ps_s = psum_s.tile([1, cfg.h], F32, name="ps_s", tag="ps_s")

        for sup in range(cfg.nsup):
            a_sb = a_pool.tile([P, cfg.kt, cfg.l_super], BF16, name="a_sb", tag="a_sb")
            ls = sup * cfg.l_super
            a_src = trgT[b, :, ls:ls + cfg.l_super].rearrange(
                "(ko ki) l -> ki ko l", ki=P)
            for ko in range(cfg.kt):
                nc.sync.dma_start(a_sb[:, ko, :], a_src[:, ko, :])

            for ltl in range(cfg.ltps):
                lt = sup * cfg.ltps + ltl
                ps = psum_t.tile([P, cfg.h], F32, name="ps", tag="ps")
                for h in range(NH):
                    hs = h * 512
                    for k in range(cfg.kt):
                        nc.tensor.matmul(
                            ps[:, hs:hs + 512],
                            lhsT=a_sb[:, k, ltl * P:(ltl + 1) * P],
                            rhs=w_sb[:, k, hs:hs + 512],
                            start=(k == 0), stop=False)
                    nc.tensor.matmul(
                        ps[:, hs:hs + 512],
                        lhsT=ones_row[:],
                        rhs=fcb_sb[:, hs:hs + 512],
                        start=False, stop=True)

                t_sb = t_pool.tile([P, cfg.h], F32R, name="t_sb", tag="t_sb")
                nc.scalar.copy(t_sb[:], ps[:])
                sq = sq_pool.tile([P, cfg.h], F32, name="sq", tag="sq")
                ss = small.tile([P, 1], F32, name="ss", tag="ss")
                t_f32v = t_sb.bitcast(F32)
                nc.vector.tensor_mul(sq[:], t_f32v[:], t_f32v[:])
                nc.vector.reduce_sum(ss[:], sq[:], axis=AX.X)
                nc.scalar.sqrt(N_all[:, lt:lt + 1], ss[:])
                nc.scalar.activation(
                    E_all[:, lt:lt + 1], N_all[:, lt:lt + 1], ACTF.Exp,
                    bias=negc[:], scale=1.0)

                e_r = E_all[:, lt:lt + 1]
                t_r = t_sb
                for h in range(NH):
                    hs = h * 512
                    nc.tensor.matmul(
                        ps_s[:, hs:hs + 512],
                        lhsT=e_r,
                        rhs=t_r[:, hs:hs + 512],
                        start=(lt == 0), stop=(lt == cfg.lt - 1),
                        skip_group_check=True)

        # --- batch tail: softmax denominator + normalization ---
        ps_z = psum_z.tile([1, cfg.lt], F32, name="ps_z", tag="ps_z")
        nc.tensor.matmul(ps_z[:], lhsT=ones_col[:],
                         rhs=E_all[:], start=True, stop=True)
        z_sb = small.tile([1, cfg.lt], F32, name="z_sb", tag="z_sb")
        nc.scalar.copy(z_sb[:], ps_z[:])
        z1 = small.tile([1, 1], F32, name="z1", tag="z1")
        nc.vector.reduce_sum(z1[:], z_sb[:], axis=AX.X)
        rz = small.tile([1, 1], F32, name="rz", tag="rz")
        nc.vector.reciprocal(rz[:], z1[:])
        s_sb = small.tile([1, cfg.h], F32, name="s_sb", tag="s_sb")
        nc.scalar.activation(s_sb[:], ps_s[:], ACTF.Copy, scale=rz[:])
        nc.sync.dma_start(summ_o[b:b + 1, :], s_sb[:])
        nc.sync.dma_start(
            norm_o[b:b + 1, :].rearrange("o (t p) -> p (o t)", p=P), N_all[:])


def build_nc(cfg: Cfg):
    from contextlib import ExitStack
    nc = bacc.Bacc("TRN2", target_bir_lowering=False, debug=False)
    with tile.TileContext(nc) as tc:
        with ExitStack() as ctx:
            emit_kernel(tc, cfg, ctx)
    nc.compile()
    return nc


_NC_CACHE = {}


def _get_nc(cfg_key=None):
    if cfg_key not in _NC_CACHE:
        _NC_CACHE[cfg_key] = build_nc(Cfg())
    return _NC_CACHE[cfg_key]


def prep_inputs(trg, fc_w, fc_b):
    """Host-side shard + layout prep. Returns per-core input maps."""
    bf16 = ml_dtypes.bfloat16
    # [B, L, K] -> [B, K, L], cast bf16
    trgT = np.ascontiguousarray(np.transpose(trg, (0, 2, 1))).astype(bf16)
    fcwT = np.ascontiguousarray(fc_w.T).astype(bf16)   # [K, H]
    fcb = np.ascontiguousarray(fc_b.reshape(1, H)).astype(bf16)
    in_maps = []
    for c in range(NCORES):
        in_maps.append({
            "trgT": trgT[c * BPC:(c + 1) * BPC],
            "fcwT": fcwT,
            "fcb": fcb,
        })
    return in_maps


def kernel(trg, src, fc_w, fc_b):
    nc = _get_nc()
    in_maps = prep_inputs(np.asarray(trg), np.asarray(fc_w), np.asarray(fc_b))
    res = run_bass_kernel_spmd(nc, in_maps, core_ids=list(range(NCORES)))
    summ = np.concatenate([r["summ"] for r in res.results], axis=0)
    norm1 = np.concatenate([r["norm1"] for r in res.results], axis=0)
    return summ.astype(np.float32), norm1.astype(np.float32)


# revision 12
# speedup vs baseline: 1.0646x; 1.0646x over previous
"""Trainium2 Bass kernel for nn_Decoder_10222022164898.

Math (reference):
    t      = trg @ fc_w.T + fc_b          # [B, L, H]
    norm1  = ||t||_2 over H               # [B, L]
    w      = softmax(norm1, axis=L)       # [B, L]
    summ   = sum_l w[l] * t[l, :]         # [B, H]
    return (summ, norm1)

Strategy: pure data-parallel over batch B=32 across 8 NeuronCores (4
batches/core, softmax is over L which stays core-local -> no collectives).

Per core, single pass over L:
  - host pre-transposes trg -> trgT [b, K, L] (bf16) so the contraction dim K
    lands on SBUF partitions with natural DMA.
  - per 128-row l-tile: PE accumulates t = trgT_tile.T @ fc_wT into PSUM over
    24 k-tiles (bf16), + one K=1 matmul ones(1x128) x fc_b for the bias.
  - ACT copies PSUM->SBUF (fp32 t), DVE does fused square+reduce straight
    from PSUM -> sumsq, ACT sqrt -> norm1, ACT exp(norm1-32) -> e.
  - PE accumulates e.T @ t into a persistent PSUM (pooling numerator);
    softmax normalization deferred to batch end:  summ = (sum_l e_l t_l) / Z.
  - batch end: Z via ones.T @ E matmul + reduce, reciprocal, scale, DMA out.
"""

import numpy as np
import ml_dtypes

import concourse.bass as bass
import concourse.mybir as mybir
import concourse.tile as tile
from concourse import bacc
from concourse.bass_utils import run_bass_kernel_spmd

F32 = mybir.dt.float32
BF16 = mybir.dt.bfloat16
F32R = mybir.dt.float32r
ALU = mybir.AluOpType
ACTF = mybir.ActivationFunctionType
AX = mybir.AxisListType

# Problem shapes (hardcoded; harness contract)
B, L, K, H = 32, 2048, 3072, 1024
NCORES = 8
BPC = B // NCORES          # batches per core
C_SHIFT = 32.0             # softmax stabilization shift (E||t|| = sqrt(H) = 32)


class Cfg:
    def __init__(self, bpc=BPC, l=L, k=K, h=H, l_super=512):
        assert l % l_super == 0 and l_super % 128 == 0
        assert k % 128 == 0 and h % 512 == 0
        self.bpc, self.l, self.k, self.h = bpc, l, k, h
        self.l_super = l_super
        self.kt = k // 128          # k-tiles
        self.lt = l // 128          # l-tiles per batch
        self.nsup = l // l_super    # DMA supertiles per batch
        self.ltps = l_super // 128  # l-tiles per supertile
        self.nh = h // 512          # h chunks of 512


def emit_kernel(tc: tile.TileContext, cfg: Cfg, ctx):
    nc = tc.nc
    P = 128
    NH = cfg.nh

    trgT = nc.dram_tensor("trgT", [cfg.bpc, cfg.k, cfg.l], BF16, kind="ExternalInput").ap()
    fcwT = nc.dram_tensor("fcwT", [cfg.k, cfg.h], BF16, kind="ExternalInput").ap()
    fcb = nc.dram_tensor("fcb", [1, cfg.h], BF16, kind="ExternalInput").ap()
    summ_o = nc.dram_tensor("summ", [cfg.bpc, cfg.h], F32, kind="ExternalOutput").ap()
    norm_o = nc.dram_tensor("norm1", [cfg.bpc, cfg.l], F32, kind="ExternalOutput").ap()

    const = ctx.enter_context(tc.tile_pool(name="const", bufs=1))
    a_pool = ctx.enter_context(tc.tile_pool(name="a_pool", bufs=2))
    t_pool = ctx.enter_context(tc.tile_pool(name="t_pool", bufs=3))
    sq_pool = ctx.enter_context(tc.tile_pool(name="sq_pool", bufs=2))
    stat = ctx.enter_context(tc.tile_pool(name="stat", bufs=2))
    small = ctx.enter_context(tc.tile_pool(name="small", bufs=2))
    psum_t = ctx.enter_context(tc.tile_pool(name="psum_t", bufs=2, space="PSUM"))
    psum_s = ctx.enter_context(tc.tile_pool(name="psum_s", bufs=1, space="PSUM"))
    psum_z = ctx.enter_context(tc.tile_pool(name="psum_z", bufs=1, space="PSUM"))

    # --- constants resident in SBUF ---
    # weight load: first k-chunk alone (unblocks the first matmuls), the
    # rest in a few big DMAs ordered after the first trg supertile.
    w_sb = const.tile([P, cfg.kt, cfg.h], BF16, name="w_sb", tag="w_sb")
    w_src = fcwT.rearrange("(ko ki) h -> ki ko h", ki=P)
    nc.sync.dma_start(w_sb[:, 0:1, :], w_src[:, 0:1, :])
    fcb_sb = const.tile([1, cfg.h], BF16, name="fcb_sb", tag="fcb_sb")
    nc.sync.dma_start(fcb_sb[:], fcb[:])
    ones_row = const.tile([1, P], BF16, name="ones_row", tag="ones_row")
    nc.vector.memset(ones_row[:], 1.0)
    ones_col = const.tile([P, 1], BF16, name="ones_col", tag="ones_col")
    nc.vector.memset(ones_col[:], 1.0)
    negc = const.tile([P, 1], F32, name="negc", tag="negc")
    nc.vector.memset(negc[:], -C_SHIFT)
    # fc_b replicated across partitions (via K=1 matmul), fp32 in SBUF;
    # the per-l-tile copyback then fuses bias-add on DVE.
    ps_fcb = psum_t.tile([P, cfg.h], F32, name="ps_fcb", tag="ps")
    for h in range(NH):
        nc.tensor.matmul(ps_fcb[:, h * 512:(h + 1) * 512], lhsT=ones_row[:],
                         rhs=fcb_sb[:, h * 512:(h + 1) * 512],
                         start=True, stop=True)
    fcb_rep = const.tile([P, cfg.h], F32, name="fcb_rep", tag="fcb_rep")
    nc.vector.tensor_copy(fcb_rep[:], ps_fcb[:])

    w_loaded = 1
    for b in range(cfg.bpc):
        E_all = stat.tile([P, cfg.lt], BF16, name="E_all", tag="E_all")
        N_all = stat.tile([P, cfg.lt], F32, name="N_all", tag="N_all")
        ps_s = psum_s.tile([1, cfg.h], F32, name="ps_s", tag="ps_s")

        for sup in range(cfg.nsup):
            a_sb = a_pool.tile([P, cfg.kt, cfg.l_super], BF16, name="a_sb", tag="a_sb")
            ls = sup * cfg.l_super
            a_src = trgT[b, :, ls:ls + cfg.l_super].rearrange(
                "(ko ki) l -> ki ko l", ki=P)
            nc.sync.dma_start(a_sb[:], a_src[:])
            # remaining weight chunks go out after the first supertile DMA
            while w_loaded < cfg.kt:
                n = min(8, cfg.kt - w_loaded)
                nc.sync.dma_start(w_sb[:, w_loaded:w_loaded + n, :],
                                  w_src[:, w_loaded:w_loaded + n, :])
                w_loaded += n

            for ltl in range(cfg.ltps):
                lt = sup * cfg.ltps + ltl
                ps = psum_t.tile([P, cfg.h], F32, name="ps", tag="ps")
                for h in range(NH):
                    hs = h * 512
                    for k in range(cfg.kt):
                        nc.tensor.matmul(
                            ps[:, hs:hs + 512],
                            lhsT=a_sb[:, k, ltl * P:(ltl + 1) * P],
                            rhs=w_sb[:, k, hs:hs + 512],
                            start=(k == 0), stop=(k == cfg.kt - 1))

                # copyback + bias add fused on DVE; bf16 t for the pool MM
                t_sb = t_pool.tile([P, cfg.h], BF16, name="t_sb", tag="t_sb")
                nc.vector.tensor_add(t_sb[:], ps[:], fcb_rep[:])
                sq = sq_pool.tile([P, cfg.h], F32, name="sq", tag="sq")
                ss = small.tile([P, 1], F32, name="ss", tag="ss")
                nc.vector.tensor_mul(sq[:], t_sb[:], t_sb[:])
                nc.vector.reduce_sum(ss[:], sq[:], axis=AX.X)
                nc.scalar.sqrt(N_all[:, lt:lt + 1], ss[:])
                nc.scalar.activation(
                    E_all[:, lt:lt + 1], N_all[:, lt:lt + 1], ACTF.Exp,
                    bias=negc[:], scale=1.0)

                for h in range(NH):
                    hs = h * 512
                    nc.tensor.matmul(
                        ps_s[:, hs:hs + 512],
                        lhsT=E_all[:, lt:lt + 1],
                        rhs=t_sb[:, hs:hs + 512],
                        start=(lt == 0), stop=(lt == cfg.lt - 1),
                        skip_group_check=True)

        # --- batch tail: softmax denominator + normalization ---
        ps_z = psum_z.tile([1, cfg.lt], F32, name="ps_z", tag="ps_z")
        nc.tensor.matmul(ps_z[:], lhsT=ones_col[:],
                         rhs=E_all[:], start=True, stop=True)
        z_sb = small.tile([1, cfg.lt], F32, name="z_sb", tag="z_sb")
        nc.scalar.copy(z_sb[:], ps_z[:])
        z1 = small.tile([1, 1], F32, name="z1", tag="z1")
        nc.vector.reduce_sum(z1[:], z_sb[:], axis=AX.X)
        rz = small.tile([1, 1], F32, name="rz", tag="rz")
        nc.vector.reciprocal(rz[:], z1[:])
        s_sb = small.tile([1, cfg.h], F32, name="s_sb", tag="s_sb")
        nc.scalar.activation(s_sb[:], ps_s[:], ACTF.Copy, scale=rz[:])
        nc.sync.dma_start(summ_o[b:b + 1, :], s_sb[:])
        nc.sync.dma_start(
            norm_o[b:b + 1, :].rearrange("o (t p) -> p (o t)", p=P), N_all[:])


def build_nc(cfg: Cfg):
    from contextlib import ExitStack
    nc = bacc.Bacc("TRN2", target_bir_lowering=False, debug=False)
    with tile.TileContext(nc) as tc:
        with ExitStack() as ctx:
            emit_kernel(tc, cfg, ctx)
    nc.compile()
    return nc


_NC_CACHE = {}


def _get_nc(cfg_key=None):
    if cfg_key not in _NC_CACHE:
        _NC_CACHE[cfg_key] = build_nc(Cfg())
    return _NC_CACHE[cfg_key]


def prep_inputs(trg, fc_w, fc_b):
    """Host-side shard + layout prep. Returns per-core input maps."""
    bf16 = ml_dtypes.bfloat16
    # [B, L, K] -> [B, K, L], cast bf16
    trgT = np.ascontiguousarray(np.transpose(trg, (0, 2, 1))).astype(bf16)
    fcwT = np.ascontiguousarray(fc_w.T).astype(bf16)   # [K, H]
    fcb = np.ascontiguousarray(fc_b.reshape(1, H)).astype(bf16)
    in_maps = []
    for c in range(NCORES):
        in_maps.append({
            "trgT": trgT[c * BPC:(c + 1) * BPC],
            "fcwT": fcwT,
            "fcb": fcb,
        })
    return in_maps


def kernel(trg, src, fc_w, fc_b):
    nc = _get_nc()
    in_maps = prep_inputs(np.asarray(trg), np.asarray(fc_w), np.asarray(fc_b))
    res = run_bass_kernel_spmd(nc, in_maps, core_ids=list(range(NCORES)))
    summ = np.concatenate([r["summ"] for r in res.results], axis=0)
    norm1 = np.concatenate([r["norm1"] for r in res.results], axis=0)
    return summ.astype(np.float32), norm1.astype(np.float32)


# revision 16
# speedup vs baseline: 1.0672x; 1.0024x over previous
"""Trainium2 Bass kernel for nn_Decoder_10222022164898.

Math (reference):
    t      = trg @ fc_w.T + fc_b          # [B, L, H]
    norm1  = ||t||_2 over H               # [B, L]
    w      = softmax(norm1, axis=L)       # [B, L]
    summ   = sum_l w[l] * t[l, :]         # [B, H]
    return (summ, norm1)

Strategy: pure data-parallel over batch B=32 across 8 NeuronCores (4
batches/core, softmax is over L which stays core-local -> no collectives).

Per core, single pass over L:
  - host pre-transposes trg -> trgT [b, K, L] (bf16) so the contraction dim K
    lands on SBUF partitions with natural DMA.
  - per 128-row l-tile: PE accumulates t = trgT_tile.T @ fc_wT into PSUM over
    24 k-tiles (bf16), + one K=1 matmul ones(1x128) x fc_b for the bias.
  - ACT copies PSUM->SBUF (fp32 t), DVE does fused square+reduce straight
    from PSUM -> sumsq, ACT sqrt -> norm1, ACT exp(norm1-32) -> e.
  - PE accumulates e.T @ t into a persistent PSUM (pooling numerator);
    softmax normalization deferred to batch end:  summ = (sum_l e_l t_l) / Z.
  - batch end: Z via ones.T @ E matmul + reduce, reciprocal, scale, DMA out.
"""

import numpy as np
import ml_dtypes

import concourse.bass as bass
import concourse.mybir as mybir
import concourse.tile as tile
from concourse import bacc
from concourse.bass_utils import run_bass_kernel_spmd

F32 = mybir.dt.float32
BF16 = mybir.dt.bfloat16
F32R = mybir.dt.float32r
ALU = mybir.AluOpType
ACTF = mybir.ActivationFunctionType
AX = mybir.AxisListType

# Problem shapes (hardcoded; harness contract)
B, L, K, H = 32, 2048, 3072, 1024
NCORES = 8
BPC = B // NCORES          # batches per core
C_SHIFT = 32.0             # softmax stabilization shift (E||t|| = sqrt(H) = 32)


class Cfg:
    def __init__(self, bpc=BPC, l=L, k=K, h=H, l_super=512):
        assert l % l_super == 0 and l_super % 128 == 0
        assert k % 128 == 0 and h % 512 == 0
        self.bpc, self.l, self.k, self.h = bpc, l, k, h
        self.l_super = l_super
        self.kt = k // 128          # k-tiles
        self.lt = l // 128          # l-tiles per batch
        self.nsup = l // l_super    # DMA supertiles per batch
        self.ltps = l_super // 128  # l-tiles per supertile
        self.nh = h // 512          # h chunks of 512


def emit_kernel(tc: tile.TileContext, cfg: Cfg, ctx):
    nc = tc.nc
    P = 128
    NH = cfg.nh

    trgT = nc.dram_tensor("trgT", [cfg.bpc, cfg.k, cfg.l], BF16, kind="ExternalInput").ap()
    fcwT = nc.dram_tensor("fcwT", [cfg.k, cfg.h], BF16, kind="ExternalInput").ap()
    fcb = nc.dram_tensor("fcb", [1, cfg.h], BF16, kind="ExternalInput").ap()
    summ_o = nc.dram_tensor("summ", [cfg.bpc, cfg.h], F32, kind="ExternalOutput").ap()
    norm_o = nc.dram_tensor("norm1", [cfg.bpc, cfg.l], F32, kind="ExternalOutput").ap()

    const = ctx.enter_context(tc.tile_pool(name="const", bufs=1))
    a_pool = ctx.enter_context(tc.tile_pool(name="a_pool", bufs=2))
    t_pool = ctx.enter_context(tc.tile_pool(name="t_pool", bufs=3))
    sq_pool = ctx.enter_context(tc.tile_pool(name="sq_pool", bufs=2))
    stat = ctx.enter_context(tc.tile_pool(name="stat", bufs=2))
    small = ctx.enter_context(tc.tile_pool(name="small", bufs=2))
    psum_t = ctx.enter_context(tc.tile_pool(name="psum_t", bufs=2, space="PSUM"))
    psum_s = ctx.enter_context(tc.tile_pool(name="psum_s", bufs=1, space="PSUM"))
    psum_z = ctx.enter_context(tc.tile_pool(name="psum_z", bufs=1, space="PSUM"))

    # --- constants resident in SBUF ---
    # weight load: first k-chunk alone (unblocks the first matmuls), the
    # rest in a few big DMAs ordered after the first trg supertile.
    w_sb = const.tile([P, cfg.kt, cfg.h], BF16, name="w_sb", tag="w_sb")
    w_src = fcwT.rearrange("(ko ki) h -> ki ko h", ki=P)
    nc.sync.dma_start(w_sb[:, 0:1, :], w_src[:, 0:1, :])
    fcb_sb = const.tile([1, cfg.h], BF16, name="fcb_sb", tag="fcb_sb")
    nc.sync.dma_start(fcb_sb[:], fcb[:])
    ones_row = const.tile([1, P], BF16, name="ones_row", tag="ones_row")
    nc.vector.memset(ones_row[:], 1.0)
    ones_col = const.tile([P, 1], BF16, name="ones_col", tag="ones_col")
    nc.vector.memset(ones_col[:], 1.0)
    negc = const.tile([P, 1], F32, name="negc", tag="negc")
    nc.vector.memset(negc[:], -C_SHIFT)
    ident = const.tile([P, P], F32, name="ident", tag="ident")
    from concourse.masks import make_identity
    make_identity(nc, ident[:])
    # fc_b replicated across partitions (via K=1 matmul), fp32 in SBUF;
    # the per-l-tile copyback then fuses bias-add on DVE.
    ps_fcb = psum_t.tile([P, cfg.h], F32, name="ps_fcb", tag="ps")
    for h in range(NH):
        nc.tensor.matmul(ps_fcb[:, h * 512:(h + 1) * 512], lhsT=ones_row[:],
                         rhs=fcb_sb[:, h * 512:(h + 1) * 512],
                         start=True, stop=True)
    fcb_rep = const.tile([P, cfg.h], F32, name="fcb_rep", tag="fcb_rep")
    nc.vector.tensor_copy(fcb_rep[:], ps_fcb[:])

    w_loaded = 1
    for b in range(cfg.bpc):
        E_all = stat.tile([P, cfg.lt], BF16, name="E_all", tag="E_all")
        N_all = stat.tile([P, cfg.lt], F32, name="N_all", tag="N_all")
        ps_s = psum_s.tile([1, cfg.h], F32, name="ps_s", tag="ps_s")

        for sup in range(cfg.nsup):
            a_sb = a_pool.tile([P, cfg.kt, cfg.l_super], BF16, name="a_sb", tag="a_sb")
            ls = sup * cfg.l_super
            a_src = trgT[b, :, ls:ls + cfg.l_super].rearrange(
                "(ko ki) l -> ki ko l", ki=P)
            if b == 0 and sup == 0:
                # split so the first l-tile's data lands ASAP
                nc.sync.dma_start(a_sb[:, :, 0:P], a_src[:, :, 0:P])
                nc.sync.dma_start(a_sb[:, :, P:], a_src[:, :, P:])
            else:
                nc.sync.dma_start(a_sb[:], a_src[:])
            # remaining weight chunks go out after the first supertile DMA
            while w_loaded < cfg.kt:
                n = min(8, cfg.kt - w_loaded)
                nc.sync.dma_start(w_sb[:, w_loaded:w_loaded + n, :],
                                  w_src[:, w_loaded:w_loaded + n, :])
                w_loaded += n

            for ltl in range(cfg.ltps):
                lt = sup * cfg.ltps + ltl
                ps = psum_t.tile([P, cfg.h], F32, name="ps", tag="ps")
                for k in range(cfg.kt):
                    for h in range(NH):
                        hs = h * 512
                        nc.tensor.matmul(
                            ps[:, hs:hs + 512],
                            lhsT=a_sb[:, k, ltl * P:(ltl + 1) * P],
                            rhs=w_sb[:, k, hs:hs + 512],
                            start=(k == 0), stop=(k == cfg.kt - 1))

                # copyback + bias add fused on DVE; bf16 t for the pool MM
                t_sb = t_pool.tile([P, cfg.h], BF16, name="t_sb", tag="t_sb")
                nc.vector.tensor_add(t_sb[:], ps[:], fcb_rep[:])
                sq = sq_pool.tile([P, cfg.h], F32, name="sq", tag="sq")
                ss = small.tile([P, 1], F32, name="ss", tag="ss")
                nc.vector.tensor_mul(sq[:], t_sb[:], t_sb[:])
                nc.vector.reduce_sum(ss[:], sq[:], axis=AX.X)
                nc.scalar.sqrt(N_all[:, lt:lt + 1], ss[:])
                nc.scalar.activation(
                    E_all[:, lt:lt + 1], N_all[:, lt:lt + 1], ACTF.Exp,
                    bias=negc[:], scale=1.0)

                for h in range(NH):
                    hs = h * 512
                    nc.tensor.matmul(
                        ps_s[:, hs:hs + 512],
                        lhsT=E_all[:, lt:lt + 1],
                        rhs=t_sb[:, hs:hs + 512],
                        start=(lt == 0), stop=(lt == cfg.lt - 1),
                        skip_group_check=True)

        # --- batch tail: softmax denominator + normalization ---
        ps_z = psum_z.tile([1, cfg.lt], F32, name="ps_z", tag="ps_z")
        nc.tensor.matmul(ps_z[:], lhsT=ones_col[:],
                         rhs=E_all[:], start=True, stop=True)
        z_sb = small.tile([1, cfg.lt], F32, name="z_sb", tag="z_sb")
        nc.scalar.copy(z_sb[:], ps_z[:])
        z1 = small.tile([1, 1], F32, name="z1", tag="z1")
        nc.vector.reduce_sum(z1[:], z_sb[:], axis=AX.X)
        rz = small.tile([1, 1], F32, name="rz", tag="rz")
        nc.vector.reciprocal(rz[:], z1[:])
        s_sb = small.tile([1, cfg.h], F32, name="s_sb", tag="s_sb")
        nc.scalar.activation(s_sb[:], ps_s[:], ACTF.Copy, scale=rz[:])
        nc.sync.dma_start(summ_o[b:b + 1, :], s_sb[:])
        # transpose norm1 on PE so the DRAM write is contiguous
        ps_n = psum_z.tile([cfg.lt, P], F32, name="ps_n", tag="ps_n")
        nc.tensor.transpose(ps_n[:], N_all[:], ident[:])
        n_t = small.tile([cfg.lt, P], F32, name="n_t", tag="n_t")
        nc.scalar.copy(n_t[:], ps_n[:])
        nc.sync.dma_start(
            norm_o[b:b + 1, :].rearrange("o (t p) -> t (o p)", p=P), n_t[:])


def build_nc(cfg: Cfg):
    from contextlib import ExitStack
    nc = bacc.Bacc("TRN2", target_bir_lowering=False, debug=False)
    with tile.TileContext(nc) as tc:
        with ExitStack() as ctx:
            emit_kernel(tc, cfg, ctx)
    nc.compile()
    return nc


_NC_CACHE = {}


def _get_nc(cfg_key=None):
    if cfg_key not in _NC_CACHE:
        _NC_CACHE[cfg_key] = build_nc(Cfg())
    return _NC_CACHE[cfg_key]


def prep_inputs(trg, fc_w, fc_b):
    """Host-side shard + layout prep. Returns per-core input maps."""
    bf16 = ml_dtypes.bfloat16
    # [B, L, K] -> [B, K, L], cast bf16
    trgT = np.ascontiguousarray(np.transpose(trg, (0, 2, 1))).astype(bf16)
    fcwT = np.ascontiguousarray(fc_w.T).astype(bf16)   # [K, H]
    fcb = np.ascontiguousarray(fc_b.reshape(1, H)).astype(bf16)
    in_maps = []
    for c in range(NCORES):
        in_maps.append({
            "trgT": trgT[c * BPC:(c + 1) * BPC],
            "fcwT": fcwT,
            "fcb": fcb,
        })
    return in_maps


def kernel(trg, src, fc_w, fc_b):
    nc = _get_nc()
    in_maps = prep_inputs(np.asarray(trg), np.asarray(fc_w), np.asarray(fc_b))
    res = run_bass_kernel_spmd(nc, in_maps, core_ids=list(range(NCORES)))
    summ = np.concatenate([r["summ"] for r in res.results], axis=0)
    norm1 = np.concatenate([r["norm1"] for r in res.results], axis=0)
    return summ.astype(np.float32), norm1.astype(np.float32)
